# revision 31
# baseline (speedup 1.0000x reference)
"""Trainium2 Bass kernel: GQA attention block (nn_Attention_66142496358763).

Full module: x -> (wq,wk,wv) projections -> RoPE(q,k) -> softmax(q k^T/sqrt(d)) v
(GQA: 32 q heads, 8 kv heads) -> wo projection.

Sharding (tensor-parallel over heads, 8 cores):
  core c: q heads [4c, 4c+4), kv head c, wq/wk/wv column shards, wo row shard
  -> each core emits a partial [S, DIM] output; host sums the 8 partials.

All TensorE math in fp16 (full-rate on trn2), fp32 PSUM accumulation,
softmax exp in fp32 on ScalarE. Softmax is computed without the max
subtraction (scores are O(10) here; a -4 bias inside exp keeps the fp16
P-matrix in range) and the denominator comes for free from a ones-column
appended to V inside the PV matmul. A/V transposes ride the DMA xbar.

Pipeline shape: the attention inner loop is paced by the ScalarE exp
stream (~0.6us per key tile vs ~0.45us of scores+PV work), so all other
PE work is interleaved into it as filler at key-tile granularity:
  - the WO projection of qblock qi-1 is emitted as 2 matmuls per slot
    ([128,512] PSUM group = 4 matmuls over the heads, spanning 2 slots)
    instead of a single 27us burst after each qblock;
  - quarter-3's Q1..Q3 projection units are deferred out of phase 1 and
    emitted 1-2 matmuls per slot inside qblock 0's attention.
PSUM budget: scores 2x[128,512] + wo 2x[128,512] + 4 PV accumulators
(each a private bank - two accumulation groups must never share a bank:
matmul start=True zeroes the whole bank row, not just its own region).
Input DMA is striped across the sync and gpsimd queues (weights keep
the scalar queue), and the drain's output DMAs are split across all
three queues so the final store doesn't serialize behind one ring.
"""

import numpy as np

S = 2048
DIM = 4096
HD = 128
NCORES = 8
HPC = 4          # q heads per core
QB = 512         # q block (seq block) size
NQB = S // QB    # 4
DKT = DIM // 128  # 32 contraction tiles for projections
KT = S // 128    # 16 key tiles for attention
SCALE = float(HD) ** -0.5
EXP_BIAS = -4.0

_CACHE = {}


def _build_nc():
    import concourse.bass as bass
    import concourse.tile as tile
    from concourse import bacc, mybir

    fp16 = mybir.dt.float16
    f32 = mybir.dt.float32
    AF = mybir.ActivationFunctionType

    nc = bacc.Bacc("TRN2", target_bir_lowering=False, debug=False)

    # xt/wkv/wq are host-packed so each SBUF partition-row is ONE
    # contiguous 2-8KB dram run (one fat DMA descriptor per partition);
    # the naive [DIM, S]-sliced layouts generated 512B-1KB descriptors
    # whose per-descriptor overhead wasted ~40% of HBM bandwidth in the
    # DMA-bound first quarter.
    xt_d = nc.dram_tensor("xt", [4 * 16, 128, 2, QB], fp16,
                          kind="ExternalInput").ap()
    wq_d = nc.dram_tensor("wq", [128, DKT, 512], fp16, kind="ExternalInput").ap()
    wkv_d = nc.dram_tensor("wkv", [128, DKT, 2 * HD], fp16,
                           kind="ExternalInput").ap()
    wo_d = nc.dram_tensor("wo", [HPC * HD, DIM], fp16, kind="ExternalInput").ap()
    rc_d = nc.dram_tensor("ropec", [HD, S], fp16, kind="ExternalInput").ap()
    rs_d = nc.dram_tensor("ropes", [HD, S], fp16, kind="ExternalInput").ap()
    id_d = nc.dram_tensor("ident", [HD, HD], fp16, kind="ExternalInput").ap()
    out_d = nc.dram_tensor("out", [S, DIM], fp16, kind="ExternalOutput").ap()

    wo_r = wo_d.rearrange("(h p) n -> p h n", p=128)
    out_r = out_d.rearrange("(st p) n -> st p n", p=128)

    with tile.TileContext(nc) as tc:
        with (
            tc.tile_pool(name="const", bufs=1) as const,
            tc.tile_pool(name="xtp", bufs=20) as xtp,
            tc.tile_pool(name="persist", bufs=1) as persist,
            tc.tile_pool(name="tmp", bufs=7) as tmp,
            tc.tile_pool(name="t12", bufs=4) as t12,
            tc.tile_pool(name="pt", bufs=4) as ptp,
            tc.tile_pool(name="asb", bufs=6) as asbp,
            tc.tile_pool(name="small", bufs=8) as small,
            tc.tile_pool(name="outp", bufs=4) as outp,
            tc.tile_pool(name="ps_s", bufs=2, space="PSUM") as ps_s,
        ):
            # ---- inputs split across the queues by NEED time. Quarter 0
            # is DMA-bound (it needs its chunks + all the weights), so it
            # runs as ONE 6-unit group (see below) whose weight needs
            # spread over ~43us, and the rings are laid out FIFO in that
            # need-order: sync = quarter-0 xt chunks; scalar (HWDGE; the
            # old gpsimd SWDGE wkv path stalled the first matmul ~10us) =
            # wkv/wq interleaved by j-block; gpsimd = small consts + rope
            # (rope is only needed once quarter-0's finish_units pop,
            # ~52us). The gpsimd memsets come first: warm_sb gates the PE
            # warm-up.
            warm_sb = const.tile([128, 128], fp16, tag="c_warm")
            nc.gpsimd.memset(warm_sb[:], 0.0)
            ebias_sb = const.tile([128, 1], f32, tag="c_eb")
            nc.gpsimd.memset(ebias_sb[:], EXP_BIAS)

            def xt_queue(qi, j):
                if qi == 0:
                    return nc.sync
                if qi == 1:
                    # quarter-1 chunks ride the near-empty gpsimd ring so
                    # they don't queue behind quarter 0's on sync
                    return nc.gpsimd
                return nc.sync if j % 2 == 0 else nc.gpsimd

            # xt streams as [128, 2, 512] quarter-chunks (0.25MB) so the
            # first projection matmuls start as soon as the first chunk
            # lands instead of waiting on a full 1MB tile; the first few
            # chunks load in kt-halves to start the K matmuls earlier
            def load_xt(qi, j):
                t = xtp.tile([128, 2, QB], fp16, tag="xt", name=f"xt_{qi}_{j}")
                eng = xt_queue(qi, j)
                if qi == 0 and j < 4:
                    eng.dma_start(t[:, 0], xt_d[qi * 16 + j][:, 0])
                    eng.dma_start(t[:, 1], xt_d[qi * 16 + j][:, 1])
                else:
                    eng.dma_start(t[:], xt_d[qi * 16 + j])
                return t

            # wkv0/wq0 load in halves so the first matmuls' weights
            # (subtile-dep on half 0 only) land earliest.
            wkv_c = {}
            wq_c = {}

            def load_wkv(j, eng, halves=False):
                w = const.tile([128, 8, 2 * HD], fp16, tag=f"c_wkv{j}",
                               name=f"wkv_c{j}")
                if halves:
                    eng.dma_start(w[:, 0:4], wkv_d[:, 8 * j:8 * j + 4, :])
                    eng.dma_start(w[:, 4:8],
                                  wkv_d[:, 8 * j + 4:8 * (j + 1), :])
                else:
                    eng.dma_start(w[:], wkv_d[:, 8 * j:8 * (j + 1), :])
                wkv_c[j] = w

            def load_wq(j, eng, halves=False):
                q = const.tile([128, 8, 512], fp16, tag=f"c_wq{j}", name=f"wq_c{j}")
                if halves:
                    eng.dma_start(q[:, 0:4], wq_d[:, 8 * j:8 * j + 4, :])
                    eng.dma_start(q[:, 4:8],
                                  wq_d[:, 8 * j + 4:8 * (j + 1), :])
                else:
                    eng.dma_start(q[:], wq_d[:, 8 * j:8 * (j + 1), :])
                wq_c[j] = q

            # whole weight stream on the scalar HWDGE ring in quarter-0
            # j-block need order (the packed layout makes it cheap);
            # sync = chunks, gpsimd = small consts + rope (rope is only
            # needed once quarter-0's finish_units pop, ~52us)
            load_wkv(0, nc.scalar, halves=True)
            load_wq(0, nc.scalar, halves=True)
            load_wkv(1, nc.scalar)
            load_wq(1, nc.scalar)
            load_wkv(2, nc.scalar)
            load_wq(2, nc.scalar)
            load_wkv(3, nc.scalar)
            load_wq(3, nc.scalar)
            xt_tiles = {0: []}
            for j in range(16):
                xt_tiles[0].append(load_xt(0, j))
            id_sb = const.tile([HD, HD], fp16, tag="c_id")
            nc.gpsimd.dma_start(id_sb[:], id_d[:])
            rc_sb = const.tile([HD, S], fp16, tag="c_rc")
            nc.gpsimd.dma_start(rc_sb[:], rc_d[:])
            rs_sb = const.tile([HD, S], fp16, tag="c_rs")
            nc.gpsimd.dma_start(rs_sb[:], rs_d[:])
            wo_sb = const.tile([128, HPC, DIM], fp16, tag="c_wo")

            # PE warm-up: dummy matmuls with no input deps sized to end
            # right as the first chunk + wkv half land (~10us); more would
            # delay the first real matmul (the engine runs in order).
            warm_ps = ps_s.tile([128, 512], f32, tag="ps_s", name="warm_ps")
            for _ in range(30):
                nc.tensor.matmul(
                    warm_ps[:, 0:128], warm_sb[:], warm_sb[:], start=True, stop=True
                )

            # persistent activations
            qt_sb = persist.tile([128, HPC, S], fp16, tag="p_qt")   # rope'd Q^T per head
            kt_sb = persist.tile([128, S], fp16, tag="p_kt")        # rope'd K^T
            va_sb = persist.tile([128, KT, 256], fp16, tag="p_va")  # V natural + ones col (256B-aligned rows for the xbar transpose)
            # A^T ping-pongs between two per-qblock tiles: divide writes
            # qblock qi while the WO filler reads qblock qi-1, and a shared
            # tile would serialize the transposes behind every WO read
            # (coarse-range WAR tracking)
            at_pp = [persist.tile([128, HPC, QB], fp16, tag=f"p_at{i}",
                                  name=f"at_pp{i}")
                     for i in range(2)]
            nc.gpsimd.memset(va_sb[:, :, 128:130], 1.0)

            # ---- phase 1: projections + rope, one seq-quarter at a time ----
            # unit order per quarter: K, V, Q0..Q3 (K needs only wk + first
            # chunks). Quarter 3's Q1..Q3 are deferred into qblock 0's
            # attention loop as PE filler (see below).
            pending = []

            def w_slice(kind, h, kt):
                if kind == "q":
                    return wq_c[kt // 8][:, kt % 8, h * HD:(h + 1) * HD]
                if kind == "k":
                    return wkv_c[kt // 8][:, kt % 8, 0:HD]
                return wkv_c[kt // 8][:, kt % 8, HD:2 * HD]

            def finish_unit(kind, raw, q0, u, v_eng=None):
                if kind == "v":
                    kt0 = q0 // 128
                    for j in range(4):
                        (v_eng or nc.scalar).dma_start_transpose(
                            va_sb[:, kt0 + j, 0:128],
                            raw[:, j * 128:(j + 1) * 128],
                        )
                else:
                    # rotate-half rope (head-dim pairs pre-permuted to
                    # (j, j+64) on the host): the partner operand is two
                    # contiguous 64-partition block copies on an idle DMA
                    # queue instead of a pair-swap matmul on the PE; the
                    # sin sign is folded into rs host-side.
                    swp = t12.tile([128, QB], fp16, tag="t12s", name="swp")
                    eng = v_eng or nc.scalar
                    eng.dma_start(swp[0:64, :], raw[64:128, :])
                    eng.dma_start(swp[64:128, :], raw[0:64, :])
                    t1 = t12.tile([128, QB], fp16, tag="t12")
                    nc.vector.tensor_mul(t1[:], raw[:], rc_sb[:, q0:q0 + QB])
                    t2 = t12.tile([128, QB], fp16, tag="t12")
                    nc.vector.tensor_mul(t2[:], swp[:], rs_sb[:, q0:q0 + QB])
                    if kind == "q":
                        dest = qt_sb[:, u, q0:q0 + QB]
                    else:
                        dest = kt_sb[:, q0:q0 + QB]
                    nc.vector.tensor_add(dest, t1[:], t2[:])

            def prefetch_xt(qi, j0):
                if qi >= NQB:
                    return
                lst = xt_tiles.setdefault(qi, [])
                for j in range(j0, j0 + 8):
                    if len(lst) > j:
                        continue
                    lst.append(load_xt(qi, j))

            with tc.tile_pool(name="ps_p1", bufs=6, space="PSUM") as ps_p1:
                # ---- quarter 0: one 6-unit group. Quarter 0 is DMA-bound
                # (chunks + all weights stream in while it runs), so all 6
                # units interleave over each chunk: chunk demand drops to
                # ~100GB/s and j-block j needs only (wkv_j, wq_j), matching
                # the scalar ring's FIFO order. K,V go first per j-block
                # (their weights arrive first), then the Q units in
                # kt-halves matching the wq0 half loads.
                U0 = [("k", -1), ("v", -1), ("q", 0), ("q", 1), ("q", 2), ("q", 3)]
                pss0 = [
                    ps_p1.tile([128, QB], f32, tag="ps_p1", name=f"pj0{gu}")
                    for gu in range(6)
                ]
                xt_c0 = xt_tiles[0]

                def q0_mm(gu, kt):
                    kind, h = U0[gu]
                    nc.tensor.matmul(
                        pss0[gu][:],
                        w_slice(kind, h, kt),
                        xt_c0[kt // 2][:, kt % 2, :],
                        start=(kt == 0),
                        stop=(kt == DKT - 1),
                    )

                for j in range(4):
                    if j == 2:
                        prefetch_xt(1, 0)
                    elif j == 3:
                        prefetch_xt(1, 8)
                    for gu in (0, 1):
                        for kt in range(8 * j, 8 * j + 8):
                            q0_mm(gu, kt)
                    for half in (0, 1):
                        for gu in (2, 3, 4, 5):
                            for kt in range(8 * j + 4 * half,
                                            8 * j + 4 * half + 4):
                                q0_mm(gu, kt)
                for gu, (kind, h) in enumerate(U0):
                    raw = tmp.tile([128, QB], fp16, tag="tmp")
                    nc.scalar.copy(raw[:], pss0[gu][:])
                    pending.append((kind, raw, 0, h))

                # ---- quarters 1-3: DMA-rich, original 3-unit groups ----
                GROUPS = [[("k", -1), ("v", -1), ("q", 0)], [("q", 1), ("q", 2), ("q", 3)]]
                for qi in range(1, NQB):
                    q0 = qi * QB
                    xt_c = xt_tiles[qi]
                    for gi, grp in enumerate(GROUPS):
                        if qi == NQB - 1 and gi == 1:
                            continue  # Q1..Q3 of quarter 3 deferred to attention filler
                        prefetch_xt(qi + 1, 0 if gi == 0 else 8)
                        pss = [
                            ps_p1.tile([128, QB], f32, tag="ps_p1", name=f"pj{gi}{gu}")
                            for gu in range(3)
                        ]
                        for j in range(4):
                            for gu, (kind, h) in enumerate(grp):
                                for kt in range(8 * j, 8 * j + 8):
                                    nc.tensor.matmul(
                                        pss[gu][:],
                                        w_slice(kind, h, kt),
                                        xt_c[kt // 2][:, kt % 2, :],
                                        start=(kt == 0),
                                        stop=(kt == DKT - 1),
                                    )
                            if pending:
                                finish_unit(*pending.pop(0))
                        for gu, (kind, h) in enumerate(grp):
                            raw = tmp.tile([128, QB], fp16, tag="tmp")
                            nc.scalar.copy(raw[:], pss[gu][:])
                            pending.append((kind, raw, q0, h))
                # quarter 3's K, V, Q0 finishes drain inside qblock 0's
                # first attention slots

            nc.scalar.dma_start(wo_sb[:], wo_r[:])

            # ---- phase 2+3: attention with WO / projection filler ----
            # per (qblock, head): 16 key-tile slots; each slot carries
            #   1 scores MM -> exp on ACT -> [filler MMs] -> 4 PV MMs
            # filler for qi>=1: half a WO group of qblock qi-1 (2 MMs; a
            # group = one 512-col wo chunk accumulated over the 4 heads,
            # spanning 2 slots) + its PSUM->SBUF copy at group end;
            # filler for qi==0: 1-2 MMs of the deferred quarter-3 Q units.
            with (
                tc.tile_pool(name="ps_wo", bufs=2, space="PSUM") as ps_wo,
                tc.tile_pool(name="ps_ac2", bufs=4, space="PSUM") as ps_ac2,
            ):
                # deferred projection filler ops: ("mm", u, kt) / ("fin", u)
                filler = []
                for u in range(1, HPC):
                    for kt in range(DKT):
                        filler.append(("mm", u, kt))
                    filler.append(("fin", u))
                filler_ps = {}

                def pop_filler(n):
                    while filler and (n > 0 or filler[0][0] == "fin"):
                        op = filler.pop(0)
                        if op[0] == "mm":
                            _, u, kt = op
                            if kt == 0:
                                filler_ps[u] = ps_wo.tile(
                                    [128, QB], f32, tag="ps_wo", name=f"dq{u}"
                                )
                            nc.tensor.matmul(
                                filler_ps[u][:],
                                w_slice("q", u, kt),
                                xt_tiles[NQB - 1][kt // 2][:, kt % 2, :],
                                start=(kt == 0),
                                stop=(kt == DKT - 1),
                            )
                            n -= 1
                        else:
                            _, u = op
                            raw = tmp.tile([128, QB], fp16, tag="tmp")
                            nc.scalar.copy(raw[:], filler_ps.pop(u)[:])
                            finish_unit("q", raw, (NQB - 1) * QB, u,
                                        v_eng=nc.sync)

                # WO emission: one (st, c) group = 4 MMs accumulating the 4
                # heads' A^T against one 512-wide wo chunk, then a DVE copy
                # into the st's output tile; DMA the st when its 8 chunks
                # are done. Emitted as (qsrc, g, half) 2-MM units.
                # dma_split routes drain-time halves over idle queues so the
                # final store isn't serialized on one ring.
                o_cur = {}
                wo_ps_cur = {}

                def emit_wo_half(qsrc, g, half, dma_split=None, pool=None,
                                 cast_eng=None):
                    sti = g // 8
                    c = g % 8
                    st = qsrc * 4 + sti
                    # output staging is a [128,1024] c-pair tile (not a
                    # full [128,4096] row): tiles recycle every ~2 groups
                    # so the drain can interleave all 4 st's stores
                    if c % 2 == 0 and half == 0:
                        o_cur[(st, c // 2)] = outp.tile(
                            [128, 1024], fp16, tag="outp", name=f"o{st}_{c//2}")
                    if half == 0:
                        pl, tg = pool or (ps_wo, "ps_wo")
                        wo_ps_cur[(st, c)] = pl.tile(
                            [128, 512], f32, tag=tg, name=f"wo{st}_{c}")
                    wo_ps = wo_ps_cur[(st, c)]
                    o_sb = o_cur[(st, c // 2)]
                    for hh in (0, 1) if half == 0 else (2, 3):
                        nc.tensor.matmul(
                            wo_ps[:],
                            at_pp[qsrc % 2][:, hh, sti * 128:(sti + 1) * 128],
                            wo_sb[:, hh, c * 512:(c + 1) * 512],
                            start=(hh == 0),
                            stop=(hh == HPC - 1),
                        )
                    if half == 1:
                        dst = o_sb[:, (c % 2) * 512:(c % 2 + 1) * 512]
                        if cast_eng is nc.scalar:
                            nc.scalar.copy(dst, wo_ps[:])
                        elif cast_eng is not None:
                            cast_eng.tensor_copy(dst, wo_ps[:])
                        else:
                            nc.vector.tensor_copy(dst, wo_ps[:])
                        # store in 0.25MB quarters: short per-DMA-engine
                        # tails keep the fabric responsive for the A^T
                        # transposes the next qblock's WO depends on
                        if c % 2 == 1:
                            qd = (dma_split or (nc.gpsimd,) * 4)[c // 2]
                            qd.dma_start(
                                out_r[st][:, (c - 1) * 512:(c + 1) * 512],
                                o_sb[:])

                wo_fifo = []
                # st12 rides the sync queue BEHIND the drain transposes
                # (in-order, so the transposes aren't blocked cross-queue)
                splits = {0: (nc.sync,) * 4,
                          1: (nc.gpsimd,) * 4,
                          2: (nc.gpsimd,) * 4,
                          3: (nc.scalar, nc.scalar, nc.scalar, nc.sync)}

                def divide_head(acc, qi, h):
                    # normalize (on DVE only; keeps ACT exp stream unblocked)
                    for qs in range(4):
                        av = acc[qs]
                        linv = small.tile([128, 1], f32, tag="small")
                        nc.vector.reciprocal(linv[:], av[:, 128:129])
                        a_sb = asbp.tile([128, 128], fp16, tag="asb")
                        nc.vector.tensor_scalar_mul(a_sb[:], av[:, 0:128], linv[:, 0:1])
                        if h == HPC - 1:
                            # h3's A^T feeds the NEXT qblock's WO filler (or
                            # the drain) almost immediately - a 1.3us DMA
                            # transpose there stalls the PE, so transpose on
                            # the PE itself (in program order, ~110ns) into
                            # a retired acc bank and copy out on DVE.
                            tr = ps_ac2.tile([128, 128], fp16, tag="ps_ac2",
                                             name=f"tr{qs}")
                            nc.tensor.transpose(tr[:], a_sb[:], id_sb[:])
                            nc.vector.tensor_copy(
                                at_pp[qi % 2][:, h, qs * 128:(qs + 1) * 128],
                                tr[:])
                        else:
                            nc.sync.dma_start_transpose(
                                at_pp[qi % 2][:, h, qs * 128:(qs + 1) * 128],
                                a_sb[:])

                prev = None       # (p_tile, kt, acc)
                for qi in range(NQB):
                    q0 = qi * QB
                    if qi >= 1:
                        # queue qblock qi-1's 64 halves; they only start at
                        # slot 8, by which point the last head's A^T
                        # transposes (emitted at slot 0) have drained
                        assert not wo_fifo
                        wo_fifo = [(qi - 1, g, hf) for g in range(32)
                                   for hf in range(2)]
                    for h in range(HPC):
                        acc = [
                            ps_ac2.tile([128, 132], f32, tag="ps_ac2",
                                        name=f"acc{i}")
                            for i in range(4)
                        ]
                        for kt in range(KT):
                            si = h * KT + kt
                            s_ps = ps_s.tile([128, 512], f32, tag="ps_s",
                                             name="s_ps")
                            nc.tensor.matmul(
                                s_ps[:],
                                kt_sb[:, kt * 128:(kt + 1) * 128],
                                qt_sb[:, h, q0:q0 + QB],
                                start=True,
                                stop=True,
                            )
                            p_t = ptp.tile([128, 512], fp16, tag="pt")
                            nc.scalar.activation(
                                p_t[:], s_ps[:], AF.Exp, bias=ebias_sb[:, 0:1],
                                scale=SCALE,
                            )
                            # PE filler while the exp stream catches up
                            if qi == 0:
                                if pending:
                                    finish_unit(*pending.pop(0), v_eng=nc.sync)
                                # 2 on odd slots: the head-closing slots
                                # (kt==15) get extra cover for PV(kt15)'s
                                # exp wait
                                pop_filler(2 if si % 2 == 1 else 1)
                            elif si >= 1:
                                # h3's A^T is now produced on the PE (in
                                # order), so filler can start at slot 1;
                                # 2/slot early to catch up, then 1/slot so
                                # the filler lasts the qblock; head-closing
                                # slots get 2 to cover PV(kt15)'s exp wait
                                for _ in range(
                                    2 if (si < 9 or si % 16 == 15) else 1
                                ):
                                    if wo_fifo:
                                        emit_wo_half(*wo_fifo.pop(0))
                            if prev is not None:
                                pp, pkt, pacc = prev
                                for qs in range(4):
                                    nc.tensor.matmul(
                                        pacc[qs][:, 0:129],
                                        pp[:, qs * 128:(qs + 1) * 128],
                                        va_sb[:, pkt, 0:129],
                                        start=(pkt == 0),
                                        stop=False,
                                    )
                            if kt == KT - 1:
                                if qi == NQB - 1 and h == HPC - 1:
                                    # drain prefix: h0/h1 matmuls of four
                                    # WO groups BEFORE the last divide, so
                                    # they aren't ordered behind its A^T
                                    # transpose writes (coarse RAW
                                    # tracking) and they cover exp(kt15)
                                    for g in range(4):
                                        emit_wo_half(
                                            NQB - 1, g, 0,
                                            dma_split=splits[0],
                                            pool=(ps_s, "ps_s") if g >= 2
                                            else None,
                                        )
                                # close the head in-slot: exp(kt15) has
                                # finished by now, and the divide's A^T
                                # transposes get a head start on the queue
                                for qs in range(4):
                                    nc.tensor.matmul(
                                        acc[qs][:, 0:129],
                                        p_t[:, qs * 128:(qs + 1) * 128],
                                        va_sb[:, kt, 0:129],
                                        start=False,
                                        stop=True,
                                    )
                                divide_head(acc, qi, h)
                                prev = None
                            else:
                                prev = (p_t, kt, acc)
                # drain: the remaining WO groups (qblock 3) w/ split stores
                # (the first four groups' h0/h1 halves were emitted before
                # the final divide, inside the last slot)
                # the drain's PSUM->SBUF casts alternate between DVE and
                # ScalarE (exp stream is done; GpSimd cannot read PSUM) so
                # the cast chain doesn't serialize the final stores on DVE
                assert not filler and not wo_fifo
                cast_cyc = [nc.vector, nc.scalar]

                def drain_cast():
                    cast_cyc.append(cast_cyc.pop(0))
                    return cast_cyc[-1]

                for g in range(4):
                    emit_wo_half(NQB - 1, g, 1, dma_split=splits[0],
                                 cast_eng=drain_cast())
                # remaining pairs in c-pair-major order so the four st
                # rows' stores interleave through the drain instead of
                # st3's whole 1MB landing at the end; each st keeps its
                # own store queue, and group-pairs rotate between the
                # ps_wo ring and the retired PV-acc banks (with only 2 wo
                # banks the ~680ns cast WARs a 350ns stall into every
                # pair)
                pairs = []
                for cp in range(4):
                    for sti in range(4):
                        if sti == 0 and cp < 2:
                            continue  # st0 c0-3 were the drain prefix
                        pairs.append((sti, cp))
                stq = [nc.sync, nc.gpsimd, nc.scalar, nc.gpsimd]
                for i, (sti, cp) in enumerate(pairs):
                    sp = (stq[sti],) * 4
                    pl = None if i % 3 == 0 else (ps_ac2, "ps_ac2")
                    ga = sti * 8 + 2 * cp
                    emit_wo_half(NQB - 1, ga, 0, dma_split=sp, pool=pl)
                    emit_wo_half(NQB - 1, ga + 1, 0, dma_split=sp, pool=pl)
                    emit_wo_half(NQB - 1, ga, 1, dma_split=sp,
                                 cast_eng=drain_cast())
                    emit_wo_half(NQB - 1, ga + 1, 1, dma_split=sp,
                                 cast_eng=drain_cast())

    nc.compile()
    return nc


def _get_nc():
    if "nc" not in _CACHE:
        _CACHE["nc"] = _build_nc()
    return _CACHE["nc"]


def _make_in_maps(x, freqs_cos, freqs_sin, wq, wk, wv, wo):
    x = np.asarray(x, dtype=np.float32)
    freqs_cos = np.asarray(freqs_cos, dtype=np.float32)
    freqs_sin = np.asarray(freqs_sin, dtype=np.float32)
    wq = np.asarray(wq, dtype=np.float32)
    wk = np.asarray(wk, dtype=np.float32)
    wv = np.asarray(wv, dtype=np.float32)
    wo = np.asarray(wo, dtype=np.float32)
    # pack xt so chunk (qi, j) is one [128, 2, 512] block with 2KB
    # contiguous per partition-row (fat DMA descriptors)
    xt3 = np.ascontiguousarray(x.T).astype(np.float16).reshape(32, 128, 2048)
    xt5 = xt3.reshape(16, 2, 128, 4, 512)
    xt = np.ascontiguousarray(xt5.transpose(3, 0, 2, 1, 4)).reshape(
        64, 128, 2, 512)

    def pack_w(w):
        # [DIM, N] -> [128, 32, N]: per-partition contiguous kt runs
        w = w.astype(np.float16)
        return np.ascontiguousarray(
            w.reshape(32, 128, w.shape[1]).transpose(1, 0, 2))

    # rotate-half rope layout: head-dim pairs (2j, 2j+1) are permuted to
    # (j, j+64) in wq/wk (scores are invariant under a shared q/k head-dim
    # permutation), so the kernel's swap operand is two contiguous
    # 64-partition block copies; sin's sign folds into rs.
    fcT = freqs_cos.T.astype(np.float32)
    fsT = freqs_sin.T.astype(np.float32)
    rc = np.concatenate([fcT, fcT], axis=0).astype(np.float16)
    rs = np.concatenate([-fsT, fsT], axis=0).astype(np.float16)
    perm = np.concatenate([np.arange(0, HD, 2), np.arange(1, HD, 2)])
    qperm = np.concatenate([h * HD + perm for h in range(HPC)])
    ident = np.eye(HD, dtype=np.float16)
    in_maps = []
    for c in range(NCORES):
        in_maps.append({
            "xt": xt,
            "wq": pack_w(wq[:, c * 512:(c + 1) * 512][:, qperm]),
            "wkv": pack_w(np.concatenate(
                [wk[:, c * 128:(c + 1) * 128][:, perm],
                 wv[:, c * 128:(c + 1) * 128]],
                axis=1)),
            "wo": np.ascontiguousarray(wo[c * 512:(c + 1) * 512, :]).astype(np.float16),
            "ropec": rc,
            "ropes": rs,
            "ident": ident,
        })
    return in_maps


def _run(inputs, trace=False):
    from concourse.bass_utils import run_bass_kernel_spmd

    nc = _get_nc()
    in_maps = _make_in_maps(**inputs)
    res = run_bass_kernel_spmd(nc, in_maps, core_ids=list(range(NCORES)), trace=trace)
    parts = [r["out"].astype(np.float32) for r in res.results]
    out = np.sum(np.stack(parts), axis=0)
    return out, res


def kernel(**inputs) -> np.ndarray:
    out, _ = _run(inputs, trace=False)
    return out



# revision 34
# speedup vs baseline: 1.0159x; 1.0159x over previous
"""Trainium2 Bass kernel: GQA attention block (nn_Attention_66142496358763).

Full module: x -> (wq,wk,wv) projections -> RoPE(q,k) -> softmax(q k^T/sqrt(d)) v
(GQA: 32 q heads, 8 kv heads) -> wo projection.

Sharding (tensor-parallel over heads, 8 cores):
  core c: q heads [4c, 4c+4), kv head c, wq/wk/wv column shards, wo row shard
  -> each core emits a partial [S, DIM] output; host sums the 8 partials.

All TensorE math in fp16 (full-rate on trn2), fp32 PSUM accumulation,
softmax exp in fp32 on ScalarE. Softmax is computed without the max
subtraction (scores are O(10) here; a -4 bias inside exp keeps the fp16
P-matrix in range) and the denominator comes for free from a ones-column
appended to V inside the PV matmul. A/V transposes ride the DMA xbar.

Pipeline shape: the attention inner loop is paced by the ScalarE exp
stream (~0.6us per key tile vs ~0.45us of scores+PV work), so all other
PE work is interleaved into it as filler at key-tile granularity:
  - the WO projection of qblock qi-1 is emitted as 2 matmuls per slot
    ([128,512] PSUM group = 4 matmuls over the heads, spanning 2 slots)
    instead of a single 27us burst after each qblock;
  - quarter-3's Q1..Q3 projection units are deferred out of phase 1 and
    emitted 1-2 matmuls per slot inside qblock 0's attention.
PSUM budget: scores 2x[128,512] + wo 2x[128,512] + 4 PV accumulators
(each a private bank - two accumulation groups must never share a bank:
matmul start=True zeroes the whole bank row, not just its own region).
Input DMA is striped across the sync and gpsimd queues (weights keep
the scalar queue), and the drain's output DMAs are split across all
three queues so the final store doesn't serialize behind one ring.
"""

import numpy as np

S = 2048
DIM = 4096
HD = 128
NCORES = 8
HPC = 4          # q heads per core
QB = 512         # q block (seq block) size
NQB = S // QB    # 4
DKT = DIM // 128  # 32 contraction tiles for projections
KT = S // 128    # 16 key tiles for attention
SCALE = float(HD) ** -0.5
EXP_BIAS = -4.0

_CACHE = {}


def _build_nc():
    import concourse.bass as bass
    import concourse.tile as tile
    from concourse import bacc, mybir

    fp16 = mybir.dt.float16
    f32 = mybir.dt.float32
    AF = mybir.ActivationFunctionType

    nc = bacc.Bacc("TRN2", target_bir_lowering=False, debug=False)

    # xt/wkv/wq are host-packed so each SBUF partition-row is ONE
    # contiguous 2-8KB dram run (one fat DMA descriptor per partition);
    # the naive [DIM, S]-sliced layouts generated 512B-1KB descriptors
    # whose per-descriptor overhead wasted ~40% of HBM bandwidth in the
    # DMA-bound first quarter.
    xt_d = nc.dram_tensor("xt", [4 * 16, 128, 2, QB], fp16,
                          kind="ExternalInput").ap()
    wq_d = nc.dram_tensor("wq", [128, DKT, 512], fp16, kind="ExternalInput").ap()
    wkv_d = nc.dram_tensor("wkv", [128, DKT, 2 * HD], fp16,
                           kind="ExternalInput").ap()
    wo_d = nc.dram_tensor("wo", [HPC * HD, DIM], fp16, kind="ExternalInput").ap()
    rc_d = nc.dram_tensor("ropec", [HD, S], fp16, kind="ExternalInput").ap()
    rs_d = nc.dram_tensor("ropes", [HD, S], fp16, kind="ExternalInput").ap()
    id_d = nc.dram_tensor("ident", [HD, HD], fp16, kind="ExternalInput").ap()
    out_d = nc.dram_tensor("out", [S, DIM], fp16, kind="ExternalOutput").ap()

    wo_r = wo_d.rearrange("(h p) n -> p h n", p=128)
    out_r = out_d.rearrange("(st p) n -> st p n", p=128)

    with tile.TileContext(nc) as tc:
        with (
            tc.tile_pool(name="const", bufs=1) as const,
            tc.tile_pool(name="xtp", bufs=20) as xtp,
            tc.tile_pool(name="persist", bufs=1) as persist,
            tc.tile_pool(name="tmp", bufs=7) as tmp,
            tc.tile_pool(name="t12", bufs=4) as t12,
            tc.tile_pool(name="pt", bufs=4) as ptp,
            tc.tile_pool(name="asb", bufs=6) as asbp,
            tc.tile_pool(name="small", bufs=8) as small,
            tc.tile_pool(name="outp", bufs=4) as outp,
            tc.tile_pool(name="ps_s", bufs=2, space="PSUM") as ps_s,
        ):
            # ---- inputs split across the queues by NEED time. Quarter 0
            # is DMA-bound (it needs its chunks + all the weights), so it
            # runs as ONE 6-unit group (see below) whose weight needs
            # spread over ~43us, and the rings are laid out FIFO in that
            # need-order: sync = quarter-0 xt chunks; scalar (HWDGE; the
            # old gpsimd SWDGE wkv path stalled the first matmul ~10us) =
            # wkv/wq interleaved by j-block; gpsimd = small consts + rope
            # (rope is only needed once quarter-0's finish_units pop,
            # ~52us). The gpsimd memsets come first: warm_sb gates the PE
            # warm-up.
            warm_sb = const.tile([128, 128], fp16, tag="c_warm")
            nc.gpsimd.memset(warm_sb[:], 0.0)
            ebias_sb = const.tile([128, 1], f32, tag="c_eb")
            nc.gpsimd.memset(ebias_sb[:], EXP_BIAS)

            def xt_queue(qi, j):
                if qi == 0:
                    return nc.sync
                return nc.sync if j % 2 == 0 else nc.gpsimd

            # xt streams as [128, 2, 512] quarter-chunks (0.25MB) so the
            # first projection matmuls start as soon as the first chunk
            # lands instead of waiting on a full 1MB tile
            def load_xt(qi, j):
                t = xtp.tile([128, 2, QB], fp16, tag="xt", name=f"xt_{qi}_{j}")
                xt_queue(qi, j).dma_start(t[:], xt_d[qi * 16 + j])
                return t

            # wkv0/wq0 load in halves so the first matmuls' weights
            # (subtile-dep on half 0 only) land earliest.
            wkv_c = {}
            wq_c = {}

            def load_wkv(j, eng, halves=False):
                w = const.tile([128, 8, 2 * HD], fp16, tag=f"c_wkv{j}",
                               name=f"wkv_c{j}")
                if halves:
                    eng.dma_start(w[:, 0:4], wkv_d[:, 8 * j:8 * j + 4, :])
                    eng.dma_start(w[:, 4:8],
                                  wkv_d[:, 8 * j + 4:8 * (j + 1), :])
                else:
                    eng.dma_start(w[:], wkv_d[:, 8 * j:8 * (j + 1), :])
                wkv_c[j] = w

            def load_wq(j, eng, halves=False):
                q = const.tile([128, 8, 512], fp16, tag=f"c_wq{j}", name=f"wq_c{j}")
                if halves:
                    eng.dma_start(q[:, 0:4], wq_d[:, 8 * j:8 * j + 4, :])
                    eng.dma_start(q[:, 4:8],
                                  wq_d[:, 8 * j + 4:8 * (j + 1), :])
                else:
                    eng.dma_start(q[:], wq_d[:, 8 * j:8 * (j + 1), :])
                wq_c[j] = q

            # whole weight stream on the scalar HWDGE ring in quarter-0
            # j-block need order (the packed layout makes it cheap);
            # sync = chunks, gpsimd = small consts + rope (rope is only
            # needed once quarter-0's finish_units pop, ~52us)
            # wkv1/wkv2 ride gpsimd (which then goes IDLE so sync's chunk
            # stream and scalar's weights split the HBM bandwidth two
            # ways); rope rides the scalar tail - it's only needed once
            # quarter-0's finish_units pop (~52us), and with rotate-half
            # nothing on the PE waits on it.
            load_wkv(0, nc.scalar, halves=True)
            load_wq(0, nc.scalar, halves=True)
            load_wq(1, nc.scalar)
            load_wq(2, nc.scalar)
            load_wkv(3, nc.scalar)
            load_wq(3, nc.scalar)
            rc_sb = const.tile([HD, S], fp16, tag="c_rc")
            nc.scalar.dma_start(rc_sb[:], rc_d[:])
            rs_sb = const.tile([HD, S], fp16, tag="c_rs")
            nc.scalar.dma_start(rs_sb[:], rs_d[:])
            xt_tiles = {0: []}
            for j in range(16):
                xt_tiles[0].append(load_xt(0, j))
            id_sb = const.tile([HD, HD], fp16, tag="c_id")
            nc.gpsimd.dma_start(id_sb[:], id_d[:])
            load_wkv(1, nc.gpsimd)
            load_wkv(2, nc.gpsimd)
            wo_sb = const.tile([128, HPC, DIM], fp16, tag="c_wo")

            # PE warm-up: dummy matmuls with no input deps sized to end
            # right as the first chunk + wkv half land (~10us); more would
            # delay the first real matmul (the engine runs in order).
            warm_ps = ps_s.tile([128, 512], f32, tag="ps_s", name="warm_ps")
            for _ in range(30):
                nc.tensor.matmul(
                    warm_ps[:, 0:128], warm_sb[:], warm_sb[:], start=True, stop=True
                )

            # persistent activations
            qt_sb = persist.tile([128, HPC, S], fp16, tag="p_qt")   # rope'd Q^T per head
            kt_sb = persist.tile([128, S], fp16, tag="p_kt")        # rope'd K^T
            va_sb = persist.tile([128, KT, 256], fp16, tag="p_va")  # V natural + ones col (256B-aligned rows for the xbar transpose)
            # A^T ping-pongs between two per-qblock tiles: divide writes
            # qblock qi while the WO filler reads qblock qi-1, and a shared
            # tile would serialize the transposes behind every WO read
            # (coarse-range WAR tracking)
            at_pp = [persist.tile([128, HPC, QB], fp16, tag=f"p_at{i}",
                                  name=f"at_pp{i}")
                     for i in range(2)]
            nc.gpsimd.memset(va_sb[:, :, 128:130], 1.0)

            # ---- phase 1: projections + rope, one seq-quarter at a time ----
            # unit order per quarter: K, V, Q0..Q3 (K needs only wk + first
            # chunks). Quarter 3's Q1..Q3 are deferred into qblock 0's
            # attention loop as PE filler (see below).
            pending = []

            def w_slice(kind, h, kt):
                if kind == "q":
                    return wq_c[kt // 8][:, kt % 8, h * HD:(h + 1) * HD]
                if kind == "k":
                    return wkv_c[kt // 8][:, kt % 8, 0:HD]
                return wkv_c[kt // 8][:, kt % 8, HD:2 * HD]

            def finish_unit(kind, raw, q0, u, v_eng=None):
                if kind == "v":
                    kt0 = q0 // 128
                    for j in range(4):
                        (v_eng or nc.scalar).dma_start_transpose(
                            va_sb[:, kt0 + j, 0:128],
                            raw[:, j * 128:(j + 1) * 128],
                        )
                else:
                    # rotate-half rope (head-dim pairs pre-permuted to
                    # (j, j+64) on the host): the partner operand is two
                    # contiguous 64-partition block copies on an idle DMA
                    # queue instead of a pair-swap matmul on the PE; the
                    # sin sign is folded into rs host-side.
                    swp = t12.tile([128, QB], fp16, tag="t12s", name="swp")
                    eng = v_eng or nc.scalar
                    eng.dma_start(swp[0:64, :], raw[64:128, :])
                    eng.dma_start(swp[64:128, :], raw[0:64, :])
                    t1 = t12.tile([128, QB], fp16, tag="t12")
                    nc.vector.tensor_mul(t1[:], raw[:], rc_sb[:, q0:q0 + QB])
                    t2 = t12.tile([128, QB], fp16, tag="t12")
                    nc.vector.tensor_mul(t2[:], swp[:], rs_sb[:, q0:q0 + QB])
                    if kind == "q":
                        dest = qt_sb[:, u, q0:q0 + QB]
                    else:
                        dest = kt_sb[:, q0:q0 + QB]
                    nc.vector.tensor_add(dest, t1[:], t2[:])

            def prefetch_xt(qi, j0):
                if qi >= NQB:
                    return
                lst = xt_tiles.setdefault(qi, [])
                for j in range(j0, j0 + 8):
                    if len(lst) > j:
                        continue
                    lst.append(load_xt(qi, j))

            with tc.tile_pool(name="ps_p1", bufs=6, space="PSUM") as ps_p1:
                # ---- quarter 0: one 6-unit group. Quarter 0 is DMA-bound
                # (chunks + all weights stream in while it runs), so all 6
                # units interleave over each chunk: chunk demand drops to
                # ~100GB/s and j-block j needs only (wkv_j, wq_j), matching
                # the scalar ring's FIFO order. K,V go first per j-block
                # (their weights arrive first), then the Q units in
                # kt-halves matching the wq0 half loads.
                U0 = [("k", -1), ("v", -1), ("q", 0), ("q", 1), ("q", 2), ("q", 3)]
                pss0 = [
                    ps_p1.tile([128, QB], f32, tag="ps_p1", name=f"pj0{gu}")
                    for gu in range(6)
                ]
                xt_c0 = xt_tiles[0]

                def q0_mm(gu, kt):
                    kind, h = U0[gu]
                    nc.tensor.matmul(
                        pss0[gu][:],
                        w_slice(kind, h, kt),
                        xt_c0[kt // 2][:, kt % 2, :],
                        start=(kt == 0),
                        stop=(kt == DKT - 1),
                    )

                for j in range(4):
                    if j == 2:
                        prefetch_xt(1, 0)
                    elif j == 3:
                        prefetch_xt(1, 8)
                    for gu in (0, 1):
                        for kt in range(8 * j, 8 * j + 8):
                            q0_mm(gu, kt)
                    for half in (0, 1):
                        for gu in (2, 3, 4, 5):
                            for kt in range(8 * j + 4 * half,
                                            8 * j + 4 * half + 4):
                                q0_mm(gu, kt)
                for gu, (kind, h) in enumerate(U0):
                    raw = tmp.tile([128, QB], fp16, tag="tmp")
                    nc.scalar.copy(raw[:], pss0[gu][:])
                    pending.append((kind, raw, 0, h))

                # ---- quarters 1-3: DMA-rich, original 3-unit groups ----
                GROUPS = [[("k", -1), ("v", -1), ("q", 0)], [("q", 1), ("q", 2), ("q", 3)]]
                for qi in range(1, NQB):
                    q0 = qi * QB
                    xt_c = xt_tiles[qi]
                    for gi, grp in enumerate(GROUPS):
                        if qi == NQB - 1 and gi == 1:
                            continue  # Q1..Q3 of quarter 3 deferred to attention filler
                        prefetch_xt(qi + 1, 0 if gi == 0 else 8)
                        pss = [
                            ps_p1.tile([128, QB], f32, tag="ps_p1", name=f"pj{gi}{gu}")
                            for gu in range(3)
                        ]
                        for j in range(4):
                            for gu, (kind, h) in enumerate(grp):
                                for kt in range(8 * j, 8 * j + 8):
                                    nc.tensor.matmul(
                                        pss[gu][:],
                                        w_slice(kind, h, kt),
                                        xt_c[kt // 2][:, kt % 2, :],
                                        start=(kt == 0),
                                        stop=(kt == DKT - 1),
                                    )
                            if pending:
                                finish_unit(*pending.pop(0))
                        for gu, (kind, h) in enumerate(grp):
                            raw = tmp.tile([128, QB], fp16, tag="tmp")
                            nc.scalar.copy(raw[:], pss[gu][:])
                            pending.append((kind, raw, q0, h))
                # quarter 3's K, V, Q0 finishes drain inside qblock 0's
                # first attention slots

            nc.scalar.dma_start(wo_sb[:], wo_r[:])

            # ---- phase 2+3: attention with WO / projection filler ----
            # per (qblock, head): 16 key-tile slots; each slot carries
            #   1 scores MM -> exp on ACT -> [filler MMs] -> 4 PV MMs
            # filler for qi>=1: half a WO group of qblock qi-1 (2 MMs; a
            # group = one 512-col wo chunk accumulated over the 4 heads,
            # spanning 2 slots) + its PSUM->SBUF copy at group end;
            # filler for qi==0: 1-2 MMs of the deferred quarter-3 Q units.
            with (
                tc.tile_pool(name="ps_wo", bufs=2, space="PSUM") as ps_wo,
                tc.tile_pool(name="ps_ac2", bufs=4, space="PSUM") as ps_ac2,
            ):
                # deferred projection filler ops: ("mm", u, kt) / ("fin", u)
                filler = []
                for u in range(1, HPC):
                    for kt in range(DKT):
                        filler.append(("mm", u, kt))
                    filler.append(("fin", u))
                filler_ps = {}

                def pop_filler(n):
                    while filler and (n > 0 or filler[0][0] == "fin"):
                        op = filler.pop(0)
                        if op[0] == "mm":
                            _, u, kt = op
                            if kt == 0:
                                filler_ps[u] = ps_wo.tile(
                                    [128, QB], f32, tag="ps_wo", name=f"dq{u}"
                                )
                            nc.tensor.matmul(
                                filler_ps[u][:],
                                w_slice("q", u, kt),
                                xt_tiles[NQB - 1][kt // 2][:, kt % 2, :],
                                start=(kt == 0),
                                stop=(kt == DKT - 1),
                            )
                            n -= 1
                        else:
                            _, u = op
                            raw = tmp.tile([128, QB], fp16, tag="tmp")
                            nc.scalar.copy(raw[:], filler_ps.pop(u)[:])
                            finish_unit("q", raw, (NQB - 1) * QB, u,
                                        v_eng=nc.sync)

                # WO emission: one (st, c) group = 4 MMs accumulating the 4
                # heads' A^T against one 512-wide wo chunk, then a DVE copy
                # into the st's output tile; DMA the st when its 8 chunks
                # are done. Emitted as (qsrc, g, half) 2-MM units.
                # dma_split routes drain-time halves over idle queues so the
                # final store isn't serialized on one ring.
                o_cur = {}
                wo_ps_cur = {}

                def emit_wo_half(qsrc, g, half, dma_split=None, pool=None,
                                 cast_eng=None):
                    sti = g // 8
                    c = g % 8
                    st = qsrc * 4 + sti
                    # output staging is a [128,1024] c-pair tile (not a
                    # full [128,4096] row): tiles recycle every ~2 groups
                    # so the drain can interleave all 4 st's stores
                    if c % 2 == 0 and half == 0:
                        o_cur[(st, c // 2)] = outp.tile(
                            [128, 1024], fp16, tag="outp", name=f"o{st}_{c//2}")
                    if half == 0:
                        pl, tg = pool or (ps_wo, "ps_wo")
                        wo_ps_cur[(st, c)] = pl.tile(
                            [128, 512], f32, tag=tg, name=f"wo{st}_{c}")
                    wo_ps = wo_ps_cur[(st, c)]
                    o_sb = o_cur[(st, c // 2)]
                    for hh in (0, 1) if half == 0 else (2, 3):
                        nc.tensor.matmul(
                            wo_ps[:],
                            at_pp[qsrc % 2][:, hh, sti * 128:(sti + 1) * 128],
                            wo_sb[:, hh, c * 512:(c + 1) * 512],
                            start=(hh == 0),
                            stop=(hh == HPC - 1),
                        )
                    if half == 1:
                        dst = o_sb[:, (c % 2) * 512:(c % 2 + 1) * 512]
                        if cast_eng is nc.scalar:
                            nc.scalar.copy(dst, wo_ps[:])
                        elif cast_eng is not None:
                            cast_eng.tensor_copy(dst, wo_ps[:])
                        else:
                            nc.vector.tensor_copy(dst, wo_ps[:])
                        # store in 0.25MB quarters: short per-DMA-engine
                        # tails keep the fabric responsive for the A^T
                        # transposes the next qblock's WO depends on
                        if c % 2 == 1:
                            qd = (dma_split or (nc.gpsimd,) * 4)[c // 2]
                            qd.dma_start(
                                out_r[st][:, (c - 1) * 512:(c + 1) * 512],
                                o_sb[:])

                wo_fifo = []
                # st12 rides the sync queue BEHIND the drain transposes
                # (in-order, so the transposes aren't blocked cross-queue)
                splits = {0: (nc.sync,) * 4,
                          1: (nc.gpsimd,) * 4,
                          2: (nc.gpsimd,) * 4,
                          3: (nc.scalar, nc.scalar, nc.scalar, nc.sync)}

                def divide_head(acc, qi, h):
                    # normalize (on DVE only; keeps ACT exp stream unblocked)
                    for qs in range(4):
                        av = acc[qs]
                        linv = small.tile([128, 1], f32, tag="small")
                        nc.vector.reciprocal(linv[:], av[:, 128:129])
                        a_sb = asbp.tile([128, 128], fp16, tag="asb")
                        nc.vector.tensor_scalar_mul(a_sb[:], av[:, 0:128], linv[:, 0:1])
                        if h == HPC - 1:
                            # h3's A^T feeds the NEXT qblock's WO filler (or
                            # the drain) almost immediately - a 1.3us DMA
                            # transpose there stalls the PE, so transpose on
                            # the PE itself (in program order, ~110ns) into
                            # a retired acc bank and copy out on DVE.
                            tr = ps_ac2.tile([128, 128], fp16, tag="ps_ac2",
                                             name=f"tr{qs}")
                            nc.tensor.transpose(tr[:], a_sb[:], id_sb[:])
                            nc.vector.tensor_copy(
                                at_pp[qi % 2][:, h, qs * 128:(qs + 1) * 128],
                                tr[:])
                        else:
                            nc.sync.dma_start_transpose(
                                at_pp[qi % 2][:, h, qs * 128:(qs + 1) * 128],
                                a_sb[:])

                prev = None       # (p_tile, kt, acc)
                for qi in range(NQB):
                    q0 = qi * QB
                    if qi >= 1:
                        # queue qblock qi-1's 64 halves; they only start at
                        # slot 8, by which point the last head's A^T
                        # transposes (emitted at slot 0) have drained
                        assert not wo_fifo
                        wo_fifo = [(qi - 1, g, hf) for g in range(32)
                                   for hf in range(2)]
                    for h in range(HPC):
                        acc = [
                            ps_ac2.tile([128, 132], f32, tag="ps_ac2",
                                        name=f"acc{i}")
                            for i in range(4)
                        ]
                        for kt in range(KT):
                            si = h * KT + kt
                            s_ps = ps_s.tile([128, 512], f32, tag="ps_s",
                                             name="s_ps")
                            nc.tensor.matmul(
                                s_ps[:],
                                kt_sb[:, kt * 128:(kt + 1) * 128],
                                qt_sb[:, h, q0:q0 + QB],
                                start=True,
                                stop=True,
                            )
                            p_t = ptp.tile([128, 512], fp16, tag="pt")
                            nc.scalar.activation(
                                p_t[:], s_ps[:], AF.Exp, bias=ebias_sb[:, 0:1],
                                scale=SCALE,
                            )
                            # PE filler while the exp stream catches up
                            if qi == 0:
                                if pending:
                                    finish_unit(*pending.pop(0), v_eng=nc.sync)
                                # 2 on odd slots: the head-closing slots
                                # (kt==15) get extra cover for PV(kt15)'s
                                # exp wait
                                pop_filler(2 if si % 2 == 1 else 1)
                            elif si >= 1:
                                # h3's A^T is now produced on the PE (in
                                # order), so filler can start at slot 1;
                                # 2/slot early to catch up, then 1/slot so
                                # the filler lasts the qblock; head-closing
                                # slots get 2 to cover PV(kt15)'s exp wait
                                for _ in range(
                                    2 if (si < 9 or si % 16 == 15) else 1
                                ):
                                    if wo_fifo:
                                        emit_wo_half(*wo_fifo.pop(0))
                            if prev is not None:
                                pp, pkt, pacc = prev
                                for qs in range(4):
                                    nc.tensor.matmul(
                                        pacc[qs][:, 0:129],
                                        pp[:, qs * 128:(qs + 1) * 128],
                                        va_sb[:, pkt, 0:129],
                                        start=(pkt == 0),
                                        stop=False,
                                    )
                            if kt == KT - 1:
                                if qi == NQB - 1 and h == HPC - 1:
                                    # drain prefix: h0/h1 matmuls of four
                                    # WO groups BEFORE the last divide, so
                                    # they aren't ordered behind its A^T
                                    # transpose writes (coarse RAW
                                    # tracking) and they cover exp(kt15)
                                    for g in range(4):
                                        emit_wo_half(
                                            NQB - 1, g, 0,
                                            dma_split=splits[0],
                                            pool=(ps_s, "ps_s") if g >= 2
                                            else None,
                                        )
                                # close the head in-slot: exp(kt15) has
                                # finished by now, and the divide's A^T
                                # transposes get a head start on the queue
                                for qs in range(4):
                                    nc.tensor.matmul(
                                        acc[qs][:, 0:129],
                                        p_t[:, qs * 128:(qs + 1) * 128],
                                        va_sb[:, kt, 0:129],
                                        start=False,
                                        stop=True,
                                    )
                                divide_head(acc, qi, h)
                                prev = None
                            else:
                                prev = (p_t, kt, acc)
                # drain: the remaining WO groups (qblock 3) w/ split stores
                # (the first four groups' h0/h1 halves were emitted before
                # the final divide, inside the last slot)
                # the drain's PSUM->SBUF casts alternate between DVE and
                # ScalarE (exp stream is done; GpSimd cannot read PSUM) so
                # the cast chain doesn't serialize the final stores on DVE
                assert not filler and not wo_fifo
                cast_cyc = [nc.vector, nc.scalar]

                def drain_cast():
                    cast_cyc.append(cast_cyc.pop(0))
                    return cast_cyc[-1]

                for g in range(4):
                    emit_wo_half(NQB - 1, g, 1, dma_split=splits[0],
                                 cast_eng=drain_cast())
                # remaining pairs in c-pair-major order so the four st
                # rows' stores interleave through the drain instead of
                # st3's whole 1MB landing at the end; each st keeps its
                # own store queue, and group-pairs rotate between the
                # ps_wo ring and the retired PV-acc banks (with only 2 wo
                # banks the ~680ns cast WARs a 350ns stall into every
                # pair)
                pairs = []
                for cp in range(4):
                    for sti in range(4):
                        if sti == 0 and cp < 2:
                            continue  # st0 c0-3 were the drain prefix
                        pairs.append((sti, cp))
                stq = [nc.sync, nc.gpsimd, nc.scalar, nc.sync]
                for i, (sti, cp) in enumerate(pairs):
                    sp = (stq[sti],) * 4
                    pl = None if i % 3 == 0 else (ps_ac2, "ps_ac2")
                    ga = sti * 8 + 2 * cp
                    emit_wo_half(NQB - 1, ga, 0, dma_split=sp, pool=pl)
                    emit_wo_half(NQB - 1, ga + 1, 0, dma_split=sp, pool=pl)
                    emit_wo_half(NQB - 1, ga, 1, dma_split=sp,
                                 cast_eng=drain_cast())
                    emit_wo_half(NQB - 1, ga + 1, 1, dma_split=sp,
                                 cast_eng=drain_cast())

    nc.compile()
    return nc


def _get_nc():
    if "nc" not in _CACHE:
        _CACHE["nc"] = _build_nc()
    return _CACHE["nc"]


def _make_in_maps(x, freqs_cos, freqs_sin, wq, wk, wv, wo):
    x = np.asarray(x, dtype=np.float32)
    freqs_cos = np.asarray(freqs_cos, dtype=np.float32)
    freqs_sin = np.asarray(freqs_sin, dtype=np.float32)
    wq = np.asarray(wq, dtype=np.float32)
    wk = np.asarray(wk, dtype=np.float32)
    wv = np.asarray(wv, dtype=np.float32)
    wo = np.asarray(wo, dtype=np.float32)
    # pack xt so chunk (qi, j) is one [128, 2, 512] block with 2KB
    # contiguous per partition-row (fat DMA descriptors)
    xt3 = np.ascontiguousarray(x.T).astype(np.float16).reshape(32, 128, 2048)
    xt5 = xt3.reshape(16, 2, 128, 4, 512)
    xt = np.ascontiguousarray(xt5.transpose(3, 0, 2, 1, 4)).reshape(
        64, 128, 2, 512)

    def pack_w(w):
        # [DIM, N] -> [128, 32, N]: per-partition contiguous kt runs
        w = w.astype(np.float16)
        return np.ascontiguousarray(
            w.reshape(32, 128, w.shape[1]).transpose(1, 0, 2))

    # rotate-half rope layout: head-dim pairs (2j, 2j+1) are permuted to
    # (j, j+64) in wq/wk (scores are invariant under a shared q/k head-dim
    # permutation), so the kernel's swap operand is two contiguous
    # 64-partition block copies; sin's sign folds into rs.
    fcT = freqs_cos.T.astype(np.float32)
    fsT = freqs_sin.T.astype(np.float32)
    rc = np.concatenate([fcT, fcT], axis=0).astype(np.float16)
    rs = np.concatenate([-fsT, fsT], axis=0).astype(np.float16)
    perm = np.concatenate([np.arange(0, HD, 2), np.arange(1, HD, 2)])
    qperm = np.concatenate([h * HD + perm for h in range(HPC)])
    ident = np.eye(HD, dtype=np.float16)
    in_maps = []
    for c in range(NCORES):
        in_maps.append({
            "xt": xt,
            "wq": pack_w(wq[:, c * 512:(c + 1) * 512][:, qperm]),
            "wkv": pack_w(np.concatenate(
                [wk[:, c * 128:(c + 1) * 128][:, perm],
                 wv[:, c * 128:(c + 1) * 128]],
                axis=1)),
            "wo": np.ascontiguousarray(wo[c * 512:(c + 1) * 512, :]).astype(np.float16),
            "ropec": rc,
            "ropes": rs,
            "ident": ident,
        })
    return in_maps


def _run(inputs, trace=False):
    from concourse.bass_utils import run_bass_kernel_spmd

    nc = _get_nc()
    in_maps = _make_in_maps(**inputs)
    res = run_bass_kernel_spmd(nc, in_maps, core_ids=list(range(NCORES)), trace=trace)
    parts = [r["out"].astype(np.float32) for r in res.results]
    out = np.sum(np.stack(parts), axis=0)
    return out, res


def kernel(**inputs) -> np.ndarray:
    out, _ = _run(inputs, trace=False)
    return out

